# revision 1
# baseline (speedup 1.0000x reference)
"""MemEffEquivariantAttention TRN2 Bass kernel.

Sharding: 8 cores = 4 batches x 2 query-token halves (fully data-parallel,
no collectives). Each core computes, for its (batch, 256-token half):
scores -> +bias(masked) -> exp (no max; range-safe) -> p = e/Z * law ->
attn = p @ vf -> equivariant LN -> out_proj.

Device dataflow per core (all 16 heads):
  - bias lands in PSUM via identity-matmul (bf16), scores accumulate on
    top with fp32 matmuls (lhsT=qT [96,128], rhs=kT_all[h] [96,1024]);
    expanded half of kT is host-gathered from outcell_index, v expansion
    is done on device with dma_gather row-gather from HBM
  - exp: ACT with accum_out => Z row-sums for free
  - u = (e * law) * 1/Z : two DVE ops (bf16)
  - u -> uT: dma_gather SBUF-source transpose, 4 heads per gather
  - attn: bf16 matmuls accumulating attnT [96, 256] over 8 s-chunks
  - LN sumsq: DVE square+accumulate, one ones-matmul per t-block
  - out_proj: bf16 matmuls over hid chunks; per-partition scale by inv
"""
import sys
sys.path.insert(0, "/opt/trn_rl_repo")

import numpy as np
import ml_dtypes

import concourse.bacc as bacc
import concourse.tile as tile
from concourse import mybir
from concourse.bass_utils import run_bass_kernel_spmd

F32 = mybir.dt.float32
F32R = mybir.dt.float32r
BF16 = mybir.dt.bfloat16
I16 = mybir.dt.int16
AF = mybir.ActivationFunctionType
ALU = mybir.AluOpType

B, T, P, HID = 4, 512, 3, 512
HD, H = 32, 16
EXP, S = 512, 1024
TQ = 256            # query tokens per core
EPS = 1e-3
CUTOFF = 1e-5
NEG = -1e30
D = P * HD          # 96, per-head feature dim

_prog_cache = {}


def _wrap_idx(idx):
    # gpsimd wrapped layout, replicated to all 8 gpsimd cores:
    # idxs[p, s] = idx[s*16 + (p % 16)]
    n = len(idx)
    w = idx.reshape(n // 16, 16).T.astype(np.int16)
    return np.ascontiguousarray(np.tile(w, (8, 1)))


def _build_program():
    nc = bacc.Bacc("TRN2", target_bir_lowering=False, debug=False)

    qT_d = nc.dram_tensor("qT", [H, D, TQ], F32R, kind="ExternalInput").ap()
    kT_d = nc.dram_tensor("kT", [H, D, S], F32R, kind="ExternalInput").ap()
    vpk_d = nc.dram_tensor("vpk", [T, H * D], BF16, kind="ExternalInput").ap()
    bias_d = nc.dram_tensor("bias", [H, 2, 128, S], BF16, kind="ExternalInput").ap()
    law_d = nc.dram_tensor("law", [2, 128, S], BF16, kind="ExternalInput").ap()
    WT_d = nc.dram_tensor("WT", [HID, HID], BF16, kind="ExternalInput").ap()
    idv_d = nc.dram_tensor("idv", [128, 32], I16, kind="ExternalInput").ap()
    idt_d = nc.dram_tensor("idt", [128, 16], I16, kind="ExternalInput").ap()
    ones_d = nc.dram_tensor("ones96", [D, 1], F32, kind="ExternalInput").ap()
    eye_d = nc.dram_tensor("eye128", [128, 128], BF16, kind="ExternalInput").ap()
    out_d = nc.dram_tensor("out", [TQ, P, HID], F32, kind="ExternalOutput").ap()

    with tile.TileContext(nc) as tc:
        with tc.tile_pool(name="const", bufs=1) as cp, \
             tc.tile_pool(name="work", bufs=3) as wp, \
             tc.tile_pool(name="kq", bufs=4) as kq, \
             tc.tile_pool(name="ug", bufs=3) as ug, \
             tc.tile_pool(name="biasp", bufs=3) as bp, \
             tc.tile_pool(name="psw", bufs=2, space="PSUM") as psw, \
             tc.tile_pool(name="psa", bufs=2, space="PSUM") as psa, \
             tc.tile_pool(name="pss", bufs=1, space="PSUM") as pss:

            # ---- constants / preload ----
            v_t = cp.tile([128, 4, H * D], BF16, tag="v")
            vg_t = cp.tile([128, 4, H * D], BF16, tag="vg")
            law_t = cp.tile([128, 2, S], BF16, tag="law")
            WT_t = cp.tile([128, 4, HID], BF16, tag="WT")
            idv_t = cp.tile([128, 32], I16, tag="idv")
            idt_t = cp.tile([128, 16], I16, tag="idt")
            ones_t = cp.tile([D, 1], F32, tag="ones")
            eye_t = cp.tile([128, 128], BF16, tag="eye")
            X_t = cp.tile([128, P, 4, TQ], BF16, tag="X")
            eps_t = cp.tile([128, 1], F32, tag="eps")
            sqacc_t = cp.tile([D, TQ], F32, tag="sqacc")
            nc.vector.memset(eps_t[:], EPS)

            nc.sync.dma_start(out=eye_t[:], in_=eye_d)
            nc.sync.dma_start(out=idt_t[:], in_=idt_d)
            nc.sync.dma_start(out=idv_t[:], in_=idv_d)
            nc.sync.dma_start(out=law_t[:], in_=law_d.rearrange("r p s -> p r s"))

            def emit_deferred_preload():
                # needed from the first attn group onwards; issued after
                # group-0 scores so they don't delay the first matmuls
                nc.sync.dma_start(out=v_t[:],
                                  in_=vpk_d.rearrange("(c p) d -> p c d", p=128))
                nc.gpsimd.dma_gather(vg_t[:], vpk_d, idv_t[:],
                                     num_idxs=EXP, num_idxs_reg=EXP,
                                     elem_size=H * D)
                nc.sync.dma_start(out=WT_t[:],
                                  in_=WT_d.rearrange("(c p) o -> p c o", p=128))
                nc.sync.dma_start(out=ones_t[:], in_=ones_d)

            ss_ps = [pss.tile([128, 1], F32, tag=f"ss{tb}", name=f"ss{tb}")
                     for tb in range(2)]

            # ---- main loop: 4 groups of 4 heads, 2-stage software pipeline
            # (scores/exp/u for group g overlap attn for group g-1 so the PE
            # never stalls on the uT transpose-gather)
            uT_tiles = {}
            u_tiles = {}

            def emit_scores_group(g):
                for h4 in range(4):
                    u1_t = ug.tile([128, 2, S], BF16, tag=f"u1_{h4}",
                                   name=f"u1_{g}_{h4}")
                    h = 4 * g + h4
                    kT_t = kq.tile([D, S], F32R, tag="kTh", name=f"kT_{h}")
                    qT_t = kq.tile([D, TQ], F32R, tag="qTh", name=f"qT_{h}")
                    bias_t = bp.tile([128, 2, S], BF16, tag="bias",
                                     name=f"bias_{h}")
                    nc.sync.dma_start(out=kT_t[:], in_=kT_d[h])
                    nc.scalar.dma_start(out=qT_t[:], in_=qT_d[h])
                    nc.scalar.dma_start(out=bias_t[:],
                                        in_=bias_d[h].rearrange("r p s -> p r s"))
                    w_tiles = [psw.tile([128, S], F32, tag="w",
                                        name=f"w_{h}_{tt}") for tt in range(2)]
                    # all 4 bias matmuls back-to-back: identity loaded once
                    for tt in range(2):
                        for half in range(2):
                            hs = slice(half * 512, (half + 1) * 512)
                            nc.tensor.matmul(w_tiles[tt][:, hs], eye_t[:],
                                             bias_t[:, tt, hs],
                                             start=True, stop=False,
                                             skip_group_check=True)
                    # then scores: qT slice loaded once per tt
                    for tt in range(2):
                        for half in range(2):
                            hs = slice(half * 512, (half + 1) * 512)
                            nc.tensor.matmul(w_tiles[tt][:, hs],
                                             qT_t[:, tt * 128:(tt + 1) * 128],
                                             kT_t[:, hs],
                                             start=False, stop=True,
                                             skip_group_check=True)
                    for tt in range(2):
                        e_t = wp.tile([128, S], BF16, tag="e")
                        z_t = wp.tile([128, 1], F32, tag="z")
                        nc.scalar.activation(e_t[:], w_tiles[tt][:], AF.Exp,
                                             accum_out=z_t[:])
                        rz_t = wp.tile([128, 1], F32, tag="rz")
                        nc.vector.reciprocal(rz_t[:], z_t[:])
                        u0_t = wp.tile([128, S], BF16, tag="u0")
                        nc.vector.tensor_mul(u0_t[:], e_t[:], law_t[:, tt, :])
                        nc.vector.tensor_scalar_mul(u1_t[:, tt, :],
                                                    u0_t[:], rz_t[:])
                    # transpose this head's u -> uT chunks [s_local, t]
                    uT_t = ug.tile([128, 8, TQ], BF16, tag=f"uT1_{h4}",
                                   name=f"uT1_{g}_{h4}")
                    nc.gpsimd.dma_gather(uT_t[:], u1_t[:], idt_t[:],
                                         num_idxs=TQ, num_idxs_reg=TQ,
                                         elem_size=S, transpose=True,
                                         sbuf_tokens_per_rank=128,
                                         sbuf_free_dim_per_rank=2 * S)
                    uT_tiles[(g, h4)] = uT_t

            def emit_attn_group(g):
                for h4 in range(4):
                    h = 4 * g + h4
                    uT_t = uT_tiles.pop((g, h4))
                    at_ps = psa.tile([D, TQ], F32, tag="attn")
                    for sc in range(8):
                        vsrc = v_t if sc < 4 else vg_t
                        nc.tensor.matmul(at_ps[:],
                                         vsrc[:, sc % 4, h * D:(h + 1) * D],
                                         uT_t[:, sc, :],
                                         start=(sc == 0), stop=(sc == 7))

                    at_sb = wp.tile([D, TQ], BF16, tag="atsb")
                    nc.scalar.activation(at_sb[:], at_ps[:], AF.Copy)

                    # stash into X[(h%4)*32+j, p, h//4, t] for out_proj lhsT
                    for p in range(P):
                        nc.sync.dma_start(
                            out=X_t[(h % 4) * 32:(h % 4 + 1) * 32, p, h // 4, :],
                            in_=at_sb[p * 32:(p + 1) * 32, :])

                    # sumsq accumulate on DVE (f32 accumulator)
                    if h == 0:
                        nc.vector.tensor_mul(sqacc_t[:], at_sb[:], at_sb[:])
                    else:
                        sq_t = wp.tile([D, TQ], BF16, tag="sq")
                        nc.vector.tensor_mul(sq_t[:], at_sb[:], at_sb[:])
                        nc.vector.tensor_add(sqacc_t[:], sqacc_t[:], sq_t[:])

            for g in range(4):
                emit_scores_group(g)
                if g == 0:
                    emit_deferred_preload()
                if g >= 1:
                    emit_attn_group(g - 1)
            emit_attn_group(3)

            # ---- inv = 1/sqrt(mean+eps), out_proj, scale, store ----
            for tb in range(2):
                nc.tensor.matmul(ss_ps[tb][:],
                                 sqacc_t[:, tb * 128:(tb + 1) * 128],
                                 ones_t[:], start=True, stop=True)
            inv_t = []
            for tb in range(2):
                tmp_t = wp.tile([128, 1], F32, tag=f"tmp{tb}")
                nc.scalar.activation(tmp_t[:], ss_ps[tb][:], AF.Sqrt,
                                     scale=1.0 / HID, bias=eps_t[:])
                iv = wp.tile([128, 1], F32, tag=f"inv{tb}")
                nc.vector.reciprocal(iv[:], tmp_t[:])
                inv_t.append(iv)

            for p in range(P):
                for tb in range(2):
                    o_ps = psa.tile([128, HID], F32, tag="attn")
                    for ci in range(4):
                        nc.tensor.matmul(o_ps[:],
                                         X_t[:, p, ci, tb * 128:(tb + 1) * 128],
                                         WT_t[:, ci, :],
                                         start=(ci == 0), stop=(ci == 3))
                    o_sb = wp.tile([128, HID], F32, tag="osb")
                    nc.vector.tensor_scalar_mul(o_sb[:], o_ps[:], inv_t[tb][:])
                    nc.sync.dma_start(out=out_d[tb * 128:(tb + 1) * 128, p, :],
                                      in_=o_sb[:])

    nc.compile()
    return nc


def _get_program():
    if "nc" not in _prog_cache:
        _prog_cache["nc"] = _build_program()
    return _prog_cache["nc"]


def _prepare_in_maps(q, k, v, attn_bias, key_padding_mask, outcell_index,
                     local_attention_weight, expand_mask, out_proj_weight,
                     attn_ln_weight):
    q = np.asarray(q, dtype=np.float32)
    k = np.asarray(k, dtype=np.float32)
    v = np.asarray(v, dtype=np.float32)
    attn_bias = np.asarray(attn_bias, dtype=np.float32)
    kpm = np.asarray(key_padding_mask)
    idx = np.asarray(outcell_index).astype(np.int64)
    law = np.asarray(local_attention_weight, dtype=np.float32)
    emask = np.asarray(expand_mask)
    W = np.asarray(out_proj_weight, dtype=np.float32)
    lnw = np.asarray(attn_ln_weight, dtype=np.float32)

    WT = np.ascontiguousarray((W * lnw[None, :]).T)  # [hid, o], ln folded
    idt_np = _wrap_idx(np.arange(TQ, dtype=np.int16))
    ones_np = np.ones((D, 1), dtype=np.float32)
    eye_np = np.eye(128, dtype=ml_dtypes.bfloat16)

    in_maps = []
    for c in range(8):
        b, th = c // 2, c % 2
        tsl = slice(th * TQ, (th + 1) * TQ)

        qT = np.ascontiguousarray(
            q[b, tsl].reshape(TQ, P, H, HD).transpose(2, 1, 3, 0).reshape(H, D, TQ))
        kTl = k[b].reshape(T, P, H, HD).transpose(2, 1, 3, 0).reshape(H, D, T)
        kT = np.concatenate([kTl, kTl[:, :, idx[b]]], axis=2)  # [H, D, 1024]
        vpk = v[b].reshape(T, P, H, HD).transpose(0, 2, 1, 3).reshape(T, H * D)

        bias_c = np.ascontiguousarray(attn_bias[b, :, tsl, :])  # [H, 256, S]
        kpmS = np.concatenate([kpm[b], emask[b]])               # [S]
        if kpmS.any():
            bias_c[:, :, kpmS] = NEG
        cut = law[b, tsl] <= CUTOFF                             # [256, S]
        if cut.any():
            bias_c[:, cut] = NEG

        in_maps.append(dict(
            qT=qT.astype(np.float32),
            kT=np.ascontiguousarray(kT).astype(np.float32),
            vpk=vpk.astype(ml_dtypes.bfloat16),
            bias=bias_c.reshape(H, 2, 128, S).astype(ml_dtypes.bfloat16),
            law=np.ascontiguousarray(law[b, tsl].reshape(2, 128, S)).astype(
                ml_dtypes.bfloat16),
            WT=WT.astype(ml_dtypes.bfloat16),
            idv=_wrap_idx(idx[b].astype(np.int16)),
            idt=idt_np,
            ones96=ones_np,
            eye128=eye_np,
        ))
    return in_maps


def kernel(**inputs):
    in_maps = _prepare_in_maps(**inputs)
    nc = _get_program()
    res = run_bass_kernel_spmd(nc, in_maps, list(range(8)))

    out = np.empty((B, T, P, HID), dtype=np.float32)
    for c in range(8):
        b, th = c // 2, c % 2
        out[b, th * TQ:(th + 1) * TQ] = res.results[c]["out"]
    return out



# revision 22
# speedup vs baseline: 1.4970x; 1.4970x over previous
"""MemEffEquivariantAttention TRN2 Bass kernel.

Sharding: 8 cores = 4 batches x 2 query-token halves (fully data-parallel,
no collectives). Each core computes, for its (batch, 256-token half):
scores -> +bias(masked) -> exp (no max; range-safe) -> p = e/Z * law ->
attn = p @ vf -> equivariant LN -> out_proj.

Transposed dataflow (v2): scores are computed TRANSPOSED, wT[s,t], with
lhsT = kT s-chunk [96,128] and rhs = qT [96,256].  Bias (pre-transposed
on host, masks folded) lands in the same PSUM region via identity
matmul.  exp(wT) -> eT feeds the attn matmul directly (contraction over
s = partition dim), so the SBUF transpose-gather of the old dataflow is
gone.  Z = sum_s exp(w) comes from an all-ones [128,128] matmul which
broadcasts Z into every partition for free; 1/Z is applied at the
PSUM->SBUF copy of attn (DVE tensor-tensor mult).  law multiplies eT on
DVE (bf16).  q/k/bias all bf16 (validated: l2 rel ~8e-3 vs 2e-2 gate).

Per-head device schedule (software-pipelined one head deep):
  PE : 8 bias-identity mm + 8 score mm (per s-half into [128,4,256]
       PSUM), then for the previous head 8 Z-ones mm + 8 attn mm
  ACT: exp per s-half [128,1024] (no accum needed)
  DVE: law mult [128,2048], at_sb = at_psum * rz_bcast
  Pool: per-head input DMAs (cheap sequencer), v-expansion dma_gather,
       reciprocal of Z is on DVE; squares for LN sumsq on Pool
  SP : X stash (out_proj lhsT regroup), const loads, output stores
"""
import sys
sys.path.insert(0, "/opt/trn_rl_repo")

import numpy as np
import ml_dtypes

import concourse.bacc as bacc
import concourse.tile as tile
from concourse import mybir
from concourse.bass_utils import run_bass_kernel_spmd

F32 = mybir.dt.float32
BF16 = mybir.dt.bfloat16
I16 = mybir.dt.int16
AF = mybir.ActivationFunctionType
ALU = mybir.AluOpType

B, T, P, HID = 4, 512, 3, 512
HD, H = 32, 16
EXP, S = 512, 1024
TQ = 256            # query tokens per core
EPS = 1e-3
CUTOFF = 1e-5
NEG = -1e30
D = P * HD          # 96, per-head feature dim
NC = 8              # s-chunks of 128

_prog_cache = {}


def _wrap_idx(idx):
    # gpsimd wrapped layout, replicated to all 8 gpsimd cores:
    # idxs[p, s] = idx[s*16 + (p % 16)]
    n = len(idx)
    w = idx.reshape(n // 16, 16).T.astype(np.int16)
    return np.ascontiguousarray(np.tile(w, (8, 1)))


def _build_program(dbg=False):
    nc = bacc.Bacc("TRN2", target_bir_lowering=False, debug=False)

    # packed per-head load: cols [0:2048)=biasT chunks, [2048:3072)=kT,
    # [3072:3328)=qT (rows 96-127 of the kT/qT region are padding)
    KOFF, QOFF, HBW = 2048, 3072, 3328
    hbuf_d = nc.dram_tensor("hbuf", [H, 128, HBW], BF16,
                            kind="ExternalInput").ap()
    vpk_d = nc.dram_tensor("vpk", [T, H * D], BF16, kind="ExternalInput").ap()
    lawT_d = nc.dram_tensor("lawT", [128, NC, TQ], BF16,
                            kind="ExternalInput").ap()
    WT_d = nc.dram_tensor("WT", [HID, HID], BF16, kind="ExternalInput").ap()
    idv_d = nc.dram_tensor("idv", [128, 32], I16, kind="ExternalInput").ap()
    ones_d = nc.dram_tensor("ones96", [D, 1], F32, kind="ExternalInput").ap()
    eye_d = nc.dram_tensor("eye128", [128, 128], BF16, kind="ExternalInput").ap()
    out_d = nc.dram_tensor("out", [TQ, P, HID], F32, kind="ExternalOutput").ap()
    if dbg:
        dbg_e = nc.dram_tensor("dbg_e", [128, NC, TQ], BF16,
                               kind="ExternalOutput").ap()
        dbg_u = nc.dram_tensor("dbg_u", [128, NC, TQ], BF16,
                               kind="ExternalOutput").ap()
        dbg_rz = nc.dram_tensor("dbg_rz", [128, TQ], F32,
                                kind="ExternalOutput").ap()
        dbg_at = nc.dram_tensor("dbg_at", [D, TQ], BF16,
                                kind="ExternalOutput").ap()
        dbg_sq = nc.dram_tensor("dbg_sq", [D, TQ], F32,
                                kind="ExternalOutput").ap()
        dbg_vg = nc.dram_tensor("dbg_vg", [128, 4, H * D], BF16,
                                kind="ExternalOutput").ap()
        dbg_z = nc.dram_tensor("dbg_z", [128, TQ], F32,
                               kind="ExternalOutput").ap()
        dbg_w = nc.dram_tensor("dbg_w", [128, NC, TQ], F32,
                               kind="ExternalOutput").ap()
        dbg_atp = nc.dram_tensor("dbg_atp", [D, TQ], F32,
                                 kind="ExternalOutput").ap()

    with tile.TileContext(nc) as tc:
        with tc.tile_pool(name="const", bufs=1) as cp, \
             tc.tile_pool(name="kq", bufs=3) as kq, \
             tc.tile_pool(name="ew", bufs=3) as ew, \
             tc.tile_pool(name="wk", bufs=3) as wp, \
             tc.tile_pool(name="psw", bufs=3, space="PSUM") as psw, \
             tc.tile_pool(name="psa", bufs=2, space="PSUM") as psa:

            # ---- constants / preload ----
            v_t = cp.tile([128, 4, H * D], BF16, tag="v")
            vg_t = cp.tile([128, 4, H * D], BF16, tag="vg")
            law_t = cp.tile([128, NC, TQ], BF16, tag="law")
            WT_t = cp.tile([128, 4, HID], BF16, tag="WT")
            idv_t = cp.tile([128, 32], I16, tag="idv")
            ones_t = cp.tile([D, 1], F32, tag="ones")
            eye_t = cp.tile([128, 128], BF16, tag="eye")
            ones128_t = cp.tile([128, 128], BF16, tag="ones128")
            X_t = cp.tile([128, P, 4, TQ], BF16, tag="X")
            eps_t = cp.tile([128, 1], F32, tag="eps")
            sqacc_t = cp.tile([D, TQ], F32, tag="sqacc")
            nc.vector.memset(eps_t[:], EPS)
            nc.vector.memset(ones128_t[:], 1.0)

            nc.sync.dma_start(out=eye_t[:], in_=eye_d)
            nc.sync.dma_start(out=idv_t[:], in_=idv_d)
            nc.sync.dma_start(out=law_t[:], in_=lawT_d)

            def emit_deferred_preload():
                # needed from the first attn group onwards; issued after
                # head-0 scores so they don't delay the first matmuls
                nc.sync.dma_start(out=v_t[:],
                                  in_=vpk_d.rearrange("(c p) d -> p c d", p=128))
                nc.gpsimd.dma_gather(vg_t[:], vpk_d, idv_t[:],
                                     num_idxs=EXP, num_idxs_reg=EXP,
                                     elem_size=H * D)
                nc.sync.dma_start(out=WT_t[:],
                                  in_=WT_d.rearrange("(c p) o -> p c o", p=128))
                nc.sync.dma_start(out=ones_t[:], in_=ones_d)

            eT_tiles = {}
            u0_tiles = {}

            def emit_load(h):
                kqb = kq.tile([128, HBW], BF16, tag="kqb", name=f"kqb{h}")
                nc.sync.dma_start(out=kqb[:], in_=hbuf_d[h])
                return kqb

            def emit_scores(h, kqb):
                eT_t = ew.tile([128, NC, TQ], BF16, tag="eT", name=f"eT{h}")
                for half in range(2):
                    wt = psw.tile([128, 4, TQ], F32, tag="wT",
                                  name=f"wT{h}_{half}")
                    # ONE start per PSUM bank: land bias for a whole bank
                    # (2 chunks) in a single [128,512] identity matmul --
                    # a second start=True on a bank with an open group
                    # discards the open group's contents.
                    for bk in range(2):
                        c0 = half * 4 + bk * 2
                        nc.tensor.matmul(wt[:, bk * 2:bk * 2 + 2, :],
                                         eye_t[:],
                                         kqb[:, c0 * TQ:(c0 + 2) * TQ],
                                         start=True, stop=False,
                                         skip_group_check=True)
                    for c4 in range(4):
                        c = half * 4 + c4
                        nc.tensor.matmul(wt[:, c4, :],
                                         kqb[0:D, KOFF + c * 128:
                                             KOFF + (c + 1) * 128],
                                         kqb[0:D, QOFF:QOFF + TQ],
                                         start=False, stop=True,
                                         skip_group_check=True)
                    nc.scalar.activation(eT_t[:, half * 4:(half + 1) * 4, :],
                                         wt[:], AF.Exp)
                    if dbg and h == 0:
                        w_sb = wp.tile([128, 4, TQ], F32, tag="wdbg",
                                       name=f"wdbg{half}")
                        nc.scalar.activation(w_sb[:], wt[:], AF.Copy)
                        nc.sync.dma_start(
                            out=dbg_w[:, half * 4:(half + 1) * 4, :],
                            in_=w_sb[:])
                # law mult over the full head (bf16, SBUF-only)
                u0_t = ew.tile([128, NC, TQ], BF16, tag="u0", name=f"u0{h}")
                nc.vector.tensor_mul(u0_t[:], eT_t[:], law_t[:])
                eT_tiles[h] = eT_t
                u0_tiles[h] = u0_t
                if dbg and h == 0:
                    nc.sync.dma_start(out=dbg_e, in_=eT_t[:])
                    nc.sync.dma_start(out=dbg_u, in_=u0_t[:])

            def emit_attn(h):
                eT_t = eT_tiles.pop(h)
                u0_t = u0_tiles.pop(h)
                atz = psa.tile([128, 2, TQ], F32, tag="atz", name=f"atz{h}")
                # Z first so the reciprocal overlaps the attn matmuls
                for c in range(NC):
                    nc.tensor.matmul(atz[:, 1, :], ones128_t[:],
                                     eT_t[:, c, :],
                                     start=(c == 0), stop=(c == NC - 1),
                                     skip_group_check=True)
                rzb = wp.tile([128, TQ], F32, tag="rzb", name=f"rzb{h}")
                nc.vector.reciprocal_approx_fast(rzb[:], atz[:, 1, :])
                for c in range(NC):
                    vsrc = v_t if c < 4 else vg_t
                    nc.tensor.matmul(atz[0:D, 0, :],
                                     vsrc[:, c % 4, h * D:(h + 1) * D],
                                     u0_t[:, c, :],
                                     start=(c == 0), stop=(c == NC - 1),
                                     skip_group_check=True)
                at_sb = wp.tile([D, TQ], BF16, tag="atsb", name=f"at{h}")
                nc.vector.tensor_mul(at_sb[:], atz[0:D, 0, :], rzb[0:D, :])
                if dbg and h == 0:
                    nc.sync.dma_start(out=dbg_rz, in_=rzb[:])
                    nc.sync.dma_start(out=dbg_at, in_=at_sb[:])
                    nc.sync.dma_start(out=dbg_vg, in_=vg_t[:])
                    z_sb = wp.tile([128, TQ], F32, tag="zdbg")
                    nc.scalar.activation(z_sb[:], atz[:, 1, :], AF.Copy)
                    nc.sync.dma_start(out=dbg_z, in_=z_sb[:])
                    atp_sb = wp.tile([D, TQ], F32, tag="atpdbg")
                    nc.scalar.activation(atp_sb[:], atz[0:D, 0, :], AF.Copy)
                    nc.sync.dma_start(out=dbg_atp, in_=atp_sb[:])

                # stash into X[(h%4)*32+j, p, h//4, t] for out_proj lhsT
                for p in range(P):
                    eng = nc.gpsimd if p == 0 else nc.sync
                    eng.dma_start(
                        out=X_t[(h % 4) * 32:(h % 4 + 1) * 32, p, h // 4, :],
                        in_=at_sb[p * 32:(p + 1) * 32, :])

                # sumsq accumulate on DVE (f32 accumulator)
                if h == 0:
                    nc.vector.tensor_mul(sqacc_t[:], at_sb[:], at_sb[:])
                else:
                    sq_t = wp.tile([D, TQ], BF16, tag="sq", name=f"sq{h}")
                    nc.vector.tensor_mul(sq_t[:], at_sb[:], at_sb[:])
                    nc.vector.tensor_add(sqacc_t[:], sqacc_t[:], sq_t[:])

            # ---- main loop: software pipeline one head deep ----
            tiles = {0: emit_load(0)}
            tiles[1] = emit_load(1)
            for h in range(H):
                emit_scores(h, tiles.pop(h))
                if h == 0:
                    emit_deferred_preload()
                if h + 2 < H:
                    tiles[h + 2] = emit_load(h + 2)
                if h >= 1:
                    emit_attn(h - 1)
            emit_attn(H - 1)

            # ---- inv = 1/sqrt(mean+eps), out_proj, scale, store ----
            if dbg:
                nc.sync.dma_start(out=dbg_sq, in_=sqacc_t[:])
            ss_t = psa.tile([128, 2, TQ], F32, tag="atz", name="ss")
            for tb in range(2):
                nc.tensor.matmul(ss_t[:, 0, tb:tb + 1],
                                 sqacc_t[:, tb * 128:(tb + 1) * 128],
                                 ones_t[:], start=True, stop=True,
                                 skip_group_check=True)
            inv_t = []
            for tb in range(2):
                tmp_t = wp.tile([128, 1], F32, tag=f"tmp{tb}")
                nc.scalar.activation(tmp_t[:], ss_t[:, 0, tb:tb + 1], AF.Sqrt,
                                     scale=1.0 / HID, bias=eps_t[:])
                iv = wp.tile([128, 1], F32, tag=f"inv{tb}")
                nc.vector.reciprocal(iv[:], tmp_t[:])
                inv_t.append(iv)

            for p in range(P):
                for tb in range(2):
                    o_ps = psa.tile([128, 2, TQ], F32, tag="atz",
                                    name=f"o{p}_{tb}")
                    for ci in range(4):
                        nc.tensor.matmul(o_ps[:, :, :],
                                         X_t[:, p, ci, tb * 128:(tb + 1) * 128],
                                         WT_t[:, ci, :],
                                         start=(ci == 0), stop=(ci == 3),
                                         skip_group_check=True)
                    o_sb = wp.tile([128, 2, TQ], F32, tag="osb")
                    nc.vector.tensor_scalar_mul(o_sb[:, :, :], o_ps[:, :, :],
                                                inv_t[tb][:])
                    nc.sync.dma_start(out=out_d[tb * 128:(tb + 1) * 128, p, :],
                                      in_=o_sb[:])

    nc.compile()
    return nc


def _get_program():
    if "nc" not in _prog_cache:
        _prog_cache["nc"] = _build_program()
    return _prog_cache["nc"]


def _prepare_in_maps(q, k, v, attn_bias, key_padding_mask, outcell_index,
                     local_attention_weight, expand_mask, out_proj_weight,
                     attn_ln_weight):
    q = np.asarray(q, dtype=np.float32)
    k = np.asarray(k, dtype=np.float32)
    v = np.asarray(v, dtype=np.float32)
    attn_bias = np.asarray(attn_bias, dtype=np.float32)
    kpm = np.asarray(key_padding_mask)
    idx = np.asarray(outcell_index).astype(np.int64)
    law = np.asarray(local_attention_weight, dtype=np.float32)
    emask = np.asarray(expand_mask)
    W = np.asarray(out_proj_weight, dtype=np.float32)
    lnw = np.asarray(attn_ln_weight, dtype=np.float32)

    WT = np.ascontiguousarray((W * lnw[None, :]).T)  # [hid, o], ln folded
    ones_np = np.ones((D, 1), dtype=np.float32)
    eye_np = np.eye(128, dtype=ml_dtypes.bfloat16)

    in_maps = []
    for c in range(8):
        b, th = c // 2, c % 2
        tsl = slice(th * TQ, (th + 1) * TQ)

        qT = np.ascontiguousarray(
            q[b, tsl].reshape(TQ, P, H, HD).transpose(2, 1, 3, 0).reshape(H, D, TQ))
        kTl = k[b].reshape(T, P, H, HD).transpose(2, 1, 3, 0).reshape(H, D, T)
        kT = np.concatenate([kTl, kTl[:, :, idx[b]]], axis=2)  # [H, D, 1024]
        vpk = v[b].reshape(T, P, H, HD).transpose(0, 2, 1, 3).reshape(T, H * D)

        bias_c = np.ascontiguousarray(attn_bias[b, :, tsl, :])  # [H, 256, S]
        kpmS = np.concatenate([kpm[b], emask[b]])               # [S]
        if kpmS.any():
            bias_c[:, :, kpmS] = NEG
        cut = law[b, tsl] <= CUTOFF                             # [256, S]
        if cut.any():
            bias_c[:, cut] = NEG
        # transpose to [H, p=s%128, c=s//128, t]
        biasT = np.ascontiguousarray(
            bias_c.reshape(H, TQ, NC, 128).transpose(0, 3, 2, 1))
        lawT = np.ascontiguousarray(
            law[b, tsl].reshape(TQ, NC, 128).transpose(2, 1, 0))

        # packed per-head load buffer: [H, 128, 3328] bf16
        hbuf = np.zeros((H, 128, 3328), dtype=ml_dtypes.bfloat16)
        hbuf[:, :, 0:2048] = biasT.reshape(H, 128, 2048).astype(
            ml_dtypes.bfloat16)
        hbuf[:, 0:D, 2048:3072] = kT.astype(ml_dtypes.bfloat16)
        hbuf[:, 0:D, 3072:3328] = qT.astype(ml_dtypes.bfloat16)

        in_maps.append(dict(
            hbuf=hbuf,
            vpk=vpk.astype(ml_dtypes.bfloat16),
            lawT=lawT.astype(ml_dtypes.bfloat16),
            WT=WT.astype(ml_dtypes.bfloat16),
            idv=_wrap_idx(idx[b].astype(np.int16)),
            ones96=ones_np,
            eye128=eye_np,
        ))
    return in_maps


def kernel(**inputs):
    in_maps = _prepare_in_maps(**inputs)
    nc = _get_program()
    res = run_bass_kernel_spmd(nc, in_maps, list(range(8)))

    out = np.empty((B, T, P, HID), dtype=np.float32)
    for c in range(8):
        b, th = c // 2, c % 2
        out[b, th * TQ:(th + 1) * TQ] = res.results[c]["out"]
    return out


# revision 31
# speedup vs baseline: 1.6644x; 1.1118x over previous
"""MemEffEquivariantAttention TRN2 Bass kernel.

Sharding: 8 cores = 4 batches x 2 query-token halves (fully data-parallel,
no collectives). Each core computes, for its (batch, 256-token half):
scores -> +bias(masked) -> exp (no max; range-safe) -> p = e/Z * law ->
attn = p @ vf -> equivariant LN -> out_proj.

Transposed dataflow (v2): scores are computed TRANSPOSED, wT[s,t], with
lhsT = kT s-chunk [96,128] and rhs = qT [96,256].  Bias (pre-transposed
on host, masks folded) lands in the same PSUM region via identity
matmul.  exp(wT) -> eT feeds the attn matmul directly (contraction over
s = partition dim), so the SBUF transpose-gather of the old dataflow is
gone.  Z = sum_s exp(w) comes from an all-ones [128,128] matmul which
broadcasts Z into every partition for free; 1/Z is applied at the
PSUM->SBUF copy of attn (DVE tensor-tensor mult).  law multiplies eT on
DVE (bf16).  q/k/bias all bf16 (validated: l2 rel ~8e-3 vs 2e-2 gate).

Per-head device schedule (software-pipelined one head deep):
  PE : 8 bias-identity mm + 8 score mm (per s-half into [128,4,256]
       PSUM), then for the previous head 8 Z-ones mm + 8 attn mm
  ACT: exp per s-half [128,1024] (no accum needed)
  DVE: law mult [128,2048], at_sb = at_psum * rz_bcast
  Pool: per-head input DMAs (cheap sequencer), v-expansion dma_gather,
       reciprocal of Z is on DVE; squares for LN sumsq on Pool
  SP : X stash (out_proj lhsT regroup), const loads, output stores
"""
import sys
sys.path.insert(0, "/opt/trn_rl_repo")

import numpy as np
import ml_dtypes

import concourse.bacc as bacc
import concourse.tile as tile
from concourse import mybir
from concourse.bass_utils import run_bass_kernel_spmd

F32 = mybir.dt.float32
BF16 = mybir.dt.bfloat16
I16 = mybir.dt.int16
AF = mybir.ActivationFunctionType
ALU = mybir.AluOpType

B, T, P, HID = 4, 512, 3, 512
HD, H = 32, 16
EXP, S = 512, 1024
TQ = 256            # query tokens per core
EPS = 1e-3
CUTOFF = 1e-5
NEG = -1e30
D = P * HD          # 96, per-head feature dim
NC = 8              # s-chunks of 128

_prog_cache = {}


def _wrap_idx(idx):
    # gpsimd wrapped layout, replicated to all 8 gpsimd cores:
    # idxs[p, s] = idx[s*16 + (p % 16)]
    n = len(idx)
    w = idx.reshape(n // 16, 16).T.astype(np.int16)
    return np.ascontiguousarray(np.tile(w, (8, 1)))


def _build_program(dbg=False):
    nc = bacc.Bacc("TRN2", target_bir_lowering=False, debug=False)

    # packed per-head load: cols [0:2048)=biasT chunks, [2048:3072)=kT,
    # [3072:3328)=qT.  DMA'd as two transfers (bias [128 rows], kq [96
    # rows]) from different engines so they land on different DMA queues.
    KOFF, QOFF, HBW = 2048, 3072, 3328
    bias_d = nc.dram_tensor("biasP", [H, 128, KOFF], BF16,
                            kind="ExternalInput").ap()
    kq_d = nc.dram_tensor("kqP", [H, D, HBW - KOFF], BF16,
                          kind="ExternalInput").ap()
    vpk_d = nc.dram_tensor("vpk", [T, H * D], BF16, kind="ExternalInput").ap()
    lawT_d = nc.dram_tensor("lawT", [128, NC, TQ], BF16,
                            kind="ExternalInput").ap()
    WT_d = nc.dram_tensor("WT", [HID, HID], BF16, kind="ExternalInput").ap()
    idv_d = nc.dram_tensor("idv", [128, 32], I16, kind="ExternalInput").ap()
    ones_d = nc.dram_tensor("ones96", [D, 1], F32, kind="ExternalInput").ap()
    eye_d = nc.dram_tensor("eye128", [128, 128], BF16, kind="ExternalInput").ap()
    out_d = nc.dram_tensor("out", [TQ, P, HID], BF16,
                           kind="ExternalOutput").ap()
    if dbg:
        dbg_e = nc.dram_tensor("dbg_e", [128, NC, TQ], BF16,
                               kind="ExternalOutput").ap()
        dbg_u = nc.dram_tensor("dbg_u", [128, NC, TQ], BF16,
                               kind="ExternalOutput").ap()
        dbg_rz = nc.dram_tensor("dbg_rz", [128, TQ], F32,
                                kind="ExternalOutput").ap()
        dbg_at = nc.dram_tensor("dbg_at", [D, TQ], BF16,
                                kind="ExternalOutput").ap()
        dbg_sq = nc.dram_tensor("dbg_sq", [D, TQ], F32,
                                kind="ExternalOutput").ap()
        dbg_vg = nc.dram_tensor("dbg_vg", [128, 4, H * D], BF16,
                                kind="ExternalOutput").ap()
        dbg_z = nc.dram_tensor("dbg_z", [128, TQ], F32,
                               kind="ExternalOutput").ap()
        dbg_w = nc.dram_tensor("dbg_w", [128, NC, TQ], F32,
                               kind="ExternalOutput").ap()
        dbg_atp = nc.dram_tensor("dbg_atp", [D, TQ], F32,
                                 kind="ExternalOutput").ap()

    with tile.TileContext(nc) as tc:
        with tc.tile_pool(name="const", bufs=1) as cp, \
             tc.tile_pool(name="kq", bufs=3) as kq, \
             tc.tile_pool(name="ew", bufs=3) as ew, \
             tc.tile_pool(name="wk", bufs=3) as wp, \
             tc.tile_pool(name="psw", bufs=3, space="PSUM") as psw, \
             tc.tile_pool(name="psa", bufs=2, space="PSUM") as psa:

            # ---- constants / preload ----
            v_t = cp.tile([128, 4, H * D], BF16, tag="v")
            vg_t = cp.tile([128, 4, H * D], BF16, tag="vg")
            law_t = cp.tile([128, NC, TQ], BF16, tag="law")
            WT_t = cp.tile([128, 4, HID], BF16, tag="WT")
            idv_t = cp.tile([128, 32], I16, tag="idv")
            ones_t = cp.tile([D, 1], F32, tag="ones")
            eye_t = cp.tile([128, 128], BF16, tag="eye")
            ones128_t = cp.tile([128, 128], BF16, tag="ones128")
            X_t = cp.tile([128, P, 4, TQ], BF16, tag="X")
            eps_t = cp.tile([128, 1], F32, tag="eps")
            sqacc_t = cp.tile([D, TQ], F32, tag="sqacc")
            nc.vector.memset(eps_t[:], EPS)
            nc.vector.memset(ones128_t[:], 1.0)

            nc.sync.dma_start(out=eye_t[:], in_=eye_d)
            nc.sync.dma_start(out=idv_t[:], in_=idv_d)
            nc.sync.dma_start(out=law_t[:], in_=lawT_d)

            def emit_deferred_preload():
                # needed from the first attn group onwards; issued after
                # head-0 scores so they don't delay the first matmuls
                nc.sync.dma_start(out=v_t[:],
                                  in_=vpk_d.rearrange("(c p) d -> p c d", p=128))
                nc.gpsimd.dma_gather(vg_t[:], vpk_d, idv_t[:],
                                     num_idxs=EXP, num_idxs_reg=EXP,
                                     elem_size=H * D)
                nc.sync.dma_start(out=WT_t[:],
                                  in_=WT_d.rearrange("(c p) o -> p c o", p=128))
                nc.sync.dma_start(out=ones_t[:], in_=ones_d)

            eT_tiles = {}
            u0_tiles = {}

            def emit_load(h):
                kqb = kq.tile([128, HBW], BF16, tag="kqb", name=f"kqb{h}")
                beng = nc.sync if h % 2 == 0 else nc.scalar
                keng = nc.scalar if h % 2 == 0 else nc.sync
                beng.dma_start(out=kqb[:, 0:KOFF], in_=bias_d[h])
                keng.dma_start(out=kqb[0:D, KOFF:HBW], in_=kq_d[h])
                return kqb

            def emit_scores(h, kqb):
                eT_t = ew.tile([128, NC, TQ], BF16, tag="eT", name=f"eT{h}")
                for half in range(2):
                    wt = psw.tile([128, 4, TQ], F32, tag="wT",
                                  name=f"wT{h}_{half}")
                    # ONE start per PSUM bank: land bias for a whole bank
                    # (2 chunks) in a single [128,512] identity matmul --
                    # a second start=True on a bank with an open group
                    # discards the open group's contents.
                    for bk in range(2):
                        c0 = half * 4 + bk * 2
                        nc.tensor.matmul(wt[:, bk * 2:bk * 2 + 2, :],
                                         eye_t[:],
                                         kqb[:, c0 * TQ:(c0 + 2) * TQ],
                                         start=True, stop=False,
                                         skip_group_check=True)
                    for c4 in range(4):
                        c = half * 4 + c4
                        nc.tensor.matmul(wt[:, c4, :],
                                         kqb[0:D, KOFF + c * 128:
                                             KOFF + (c + 1) * 128],
                                         kqb[0:D, QOFF:QOFF + TQ],
                                         start=False, stop=True,
                                         skip_group_check=True)
                    nc.scalar.activation(eT_t[:, half * 4:(half + 1) * 4, :],
                                         wt[:], AF.Exp)
                    if dbg and h == 0:
                        w_sb = wp.tile([128, 4, TQ], F32, tag="wdbg",
                                       name=f"wdbg{half}")
                        nc.scalar.activation(w_sb[:], wt[:], AF.Copy)
                        nc.sync.dma_start(
                            out=dbg_w[:, half * 4:(half + 1) * 4, :],
                            in_=w_sb[:])
                # law mult over the full head (bf16, SBUF-only)
                u0_t = ew.tile([128, NC, TQ], BF16, tag="u0", name=f"u0{h}")
                nc.vector.tensor_mul(u0_t[:], eT_t[:], law_t[:])
                eT_tiles[h] = eT_t
                u0_tiles[h] = u0_t
                if dbg and h == 0:
                    nc.sync.dma_start(out=dbg_e, in_=eT_t[:])
                    nc.sync.dma_start(out=dbg_u, in_=u0_t[:])

            def emit_attn(h):
                eT_t = eT_tiles.pop(h)
                u0_t = u0_tiles.pop(h)
                atz = psa.tile([128, 2, TQ], F32, tag="atz", name=f"atz{h}")
                # Z first so the reciprocal overlaps the attn matmuls
                for c in range(NC):
                    nc.tensor.matmul(atz[:, 1, :], ones128_t[:],
                                     eT_t[:, c, :],
                                     start=(c == 0), stop=(c == NC - 1),
                                     skip_group_check=True)
                rzb = wp.tile([128, TQ], F32, tag="rzb", name=f"rzb{h}")
                nc.vector.reciprocal_approx_fast(rzb[:], atz[:, 1, :])
                for c in range(NC):
                    vsrc = v_t if c < 4 else vg_t
                    nc.tensor.matmul(atz[0:D, 0, :],
                                     vsrc[:, c % 4, h * D:(h + 1) * D],
                                     u0_t[:, c, :],
                                     start=(c == 0), stop=(c == NC - 1),
                                     skip_group_check=True)
                at_sb = wp.tile([D, TQ], BF16, tag="atsb", name=f"at{h}")
                nc.vector.tensor_mul(at_sb[:], atz[0:D, 0, :], rzb[0:D, :])
                if dbg and h == 0:
                    nc.sync.dma_start(out=dbg_rz, in_=rzb[:])
                    nc.sync.dma_start(out=dbg_at, in_=at_sb[:])
                    nc.sync.dma_start(out=dbg_vg, in_=vg_t[:])
                    z_sb = wp.tile([128, TQ], F32, tag="zdbg")
                    nc.scalar.activation(z_sb[:], atz[:, 1, :], AF.Copy)
                    nc.sync.dma_start(out=dbg_z, in_=z_sb[:])
                    atp_sb = wp.tile([D, TQ], F32, tag="atpdbg")
                    nc.scalar.activation(atp_sb[:], atz[0:D, 0, :], AF.Copy)
                    nc.sync.dma_start(out=dbg_atp, in_=atp_sb[:])

                # stash into X[(h%4)*32+j, p, h//4, t] for out_proj lhsT
                for p, eng in ((0, nc.gpsimd), (1, nc.sync), (2, nc.sync)):
                    eng.dma_start(
                        out=X_t[(h % 4) * 32:(h % 4 + 1) * 32, p, h // 4, :],
                        in_=at_sb[p * 32:(p + 1) * 32, :])

                # sumsq accumulate on DVE (f32 accumulator)
                if h == 0:
                    nc.vector.tensor_mul(sqacc_t[:], at_sb[:], at_sb[:])
                else:
                    sq_t = wp.tile([D, TQ], BF16, tag="sq", name=f"sq{h}")
                    nc.vector.tensor_mul(sq_t[:], at_sb[:], at_sb[:])
                    nc.vector.tensor_add(sqacc_t[:], sqacc_t[:], sq_t[:])

            # ---- main loop: software pipeline one head deep ----
            tiles = {0: emit_load(0)}
            tiles[1] = emit_load(1)
            for h in range(H):
                emit_scores(h, tiles.pop(h))
                if h == 0:
                    emit_deferred_preload()
                if h + 2 < H:
                    tiles[h + 2] = emit_load(h + 2)
                if h >= 1:
                    emit_attn(h - 1)
            emit_attn(H - 1)

            # ---- inv = 1/sqrt(mean+eps), out_proj, scale, store ----
            if dbg:
                nc.sync.dma_start(out=dbg_sq, in_=sqacc_t[:])
            ss_t = psa.tile([128, 2, TQ], F32, tag="atz", name="ss")
            for tb in range(2):
                nc.tensor.matmul(ss_t[:, 0, tb:tb + 1],
                                 sqacc_t[:, tb * 128:(tb + 1) * 128],
                                 ones_t[:], start=True, stop=True,
                                 skip_group_check=True)
            inv_t = []
            for tb in range(2):
                tmp_t = wp.tile([128, 1], F32, tag=f"tmp{tb}")
                nc.scalar.activation(tmp_t[:], ss_t[:, 0, tb:tb + 1], AF.Sqrt,
                                     scale=1.0 / HID, bias=eps_t[:])
                iv = wp.tile([128, 1], F32, tag=f"inv{tb}")
                nc.vector.reciprocal(iv[:], tmp_t[:])
                inv_t.append(iv)

            for p in range(P):
                for tb in range(2):
                    o_ps = psa.tile([128, 2, TQ], F32, tag="atz",
                                    name=f"o{p}_{tb}")
                    for ci in range(4):
                        nc.tensor.matmul(o_ps[:, :, :],
                                         X_t[:, p, ci, tb * 128:(tb + 1) * 128],
                                         WT_t[:, ci, :],
                                         start=(ci == 0), stop=(ci == 3),
                                         skip_group_check=True)
                    o_sb = wp.tile([128, 2, TQ], BF16, tag="osb")
                    nc.vector.tensor_scalar_mul(o_sb[:, :, :], o_ps[:, :, :],
                                                inv_t[tb][:])
                    nc.sync.dma_start(out=out_d[tb * 128:(tb + 1) * 128, p, :],
                                      in_=o_sb[:])

    nc.compile()
    return nc


def _get_program():
    if "nc" not in _prog_cache:
        _prog_cache["nc"] = _build_program()
    return _prog_cache["nc"]


def _prepare_in_maps(q, k, v, attn_bias, key_padding_mask, outcell_index,
                     local_attention_weight, expand_mask, out_proj_weight,
                     attn_ln_weight):
    q = np.asarray(q, dtype=np.float32)
    k = np.asarray(k, dtype=np.float32)
    v = np.asarray(v, dtype=np.float32)
    attn_bias = np.asarray(attn_bias, dtype=np.float32)
    kpm = np.asarray(key_padding_mask)
    idx = np.asarray(outcell_index).astype(np.int64)
    law = np.asarray(local_attention_weight, dtype=np.float32)
    emask = np.asarray(expand_mask)
    W = np.asarray(out_proj_weight, dtype=np.float32)
    lnw = np.asarray(attn_ln_weight, dtype=np.float32)

    WT = np.ascontiguousarray((W * lnw[None, :]).T)  # [hid, o], ln folded
    ones_np = np.ones((D, 1), dtype=np.float32)
    eye_np = np.eye(128, dtype=ml_dtypes.bfloat16)

    in_maps = []
    for c in range(8):
        b, th = c // 2, c % 2
        tsl = slice(th * TQ, (th + 1) * TQ)

        qT = np.ascontiguousarray(
            q[b, tsl].reshape(TQ, P, H, HD).transpose(2, 1, 3, 0).reshape(H, D, TQ))
        kTl = k[b].reshape(T, P, H, HD).transpose(2, 1, 3, 0).reshape(H, D, T)
        kT = np.concatenate([kTl, kTl[:, :, idx[b]]], axis=2)  # [H, D, 1024]
        vpk = v[b].reshape(T, P, H, HD).transpose(0, 2, 1, 3).reshape(T, H * D)

        bias_c = np.ascontiguousarray(attn_bias[b, :, tsl, :])  # [H, 256, S]
        kpmS = np.concatenate([kpm[b], emask[b]])               # [S]
        if kpmS.any():
            bias_c[:, :, kpmS] = NEG
        cut = law[b, tsl] <= CUTOFF                             # [256, S]
        if cut.any():
            bias_c[:, cut] = NEG
        # transpose to [H, p=s%128, c=s//128, t]
        biasT = np.ascontiguousarray(
            bias_c.reshape(H, TQ, NC, 128).transpose(0, 3, 2, 1))
        lawT = np.ascontiguousarray(
            law[b, tsl].reshape(TQ, NC, 128).transpose(2, 1, 0))

        # packed per-head load buffers
        biasP = np.ascontiguousarray(
            biasT.reshape(H, 128, 2048)).astype(ml_dtypes.bfloat16)
        kqP = np.concatenate([kT, qT], axis=2).astype(ml_dtypes.bfloat16)

        in_maps.append(dict(
            biasP=biasP,
            kqP=np.ascontiguousarray(kqP),
            vpk=vpk.astype(ml_dtypes.bfloat16),
            lawT=lawT.astype(ml_dtypes.bfloat16),
            WT=WT.astype(ml_dtypes.bfloat16),
            idv=_wrap_idx(idx[b].astype(np.int16)),
            ones96=ones_np,
            eye128=eye_np,
        ))
    return in_maps


def kernel(**inputs):
    in_maps = _prepare_in_maps(**inputs)
    nc = _get_program()
    res = run_bass_kernel_spmd(nc, in_maps, list(range(8)))

    out = np.empty((B, T, P, HID), dtype=np.float32)
    for c in range(8):
        b, th = c // 2, c % 2
        out[b, th * TQ:(th + 1) * TQ] = res.results[c]["out"].astype(
            np.float32)
    return out
